# revision 23
# baseline (speedup 1.0000x reference)
"""Multi-head attention (B=4, N=2048, C=768, H=12) on 8 TRN2 NeuronCores.

Sharding: 4 batches x 2 head-groups (6 heads each); core = 2*b + g.
Per core:
  - qT/kT [64,2048] per head and v [2048,64] per head from host-pre-transposed xT
  - flash-style attention on transposed-S tiles:
      S^T(m,n) = kT.T @ qT   (PE, bf16, two heads paired in disjoint row groups)
      P^T = exp(S^T/8)       (ACT, -> bf16)
      o^T = [v|1].T @ P^T    (PE, bf16; ones column accumulates softmax row-sums)
  - normalize columns of o^T via reciprocal + K=1 broadcast matmul
  - AllGather normalized aoT (bf16) between pair cores, one collective per
    512-column chunk so each gather + projection hides under later attention
  - each core projects full aoT onto its half of w_out columns -> y [2048,384]

Scheduling: chunk-major, head-pair-rotating attention order. The qkv GEMM +
projection GEMM work is cut into single-PSUM-group units and drained one group
per attention mj-iteration ("background work"), so the PE stream stays dense
while ACT paces the attention inner loop. proj(ci) groups drain two chunks
after chunk ci's AllGather is emitted because ships lag the chunk end by the
norm chain and the collective waits on the partner's ships (~10us on HW).
Host only concatenates the 8 column-slices (no host math).
"""

import sys

sys.path.insert(0, "/opt/trn_rl_repo")

import ml_dtypes
import numpy as np

import concourse.bass as bass
import concourse.mybir as mybir
from concourse import bacc, tile
from concourse.bass_utils import run_bass_kernel_spmd

F32 = mybir.dt.float32
BF16 = mybir.dt.bfloat16

B, N, C, H, D = 4, 2048, 768, 12, 64
G = 2               # head groups (tensor-parallel dim)
HPC = H // G        # heads per core = 6
KC = HPC * D        # per-core head width = 384
CT = C // 128       # contraction tiles over C = 6
NT = N // 128       # 128-row seq tiles = 16
SCALE = D ** -0.5


def _build():
    nc = bacc.Bacc(None, num_devices=8)

    xT_d = nc.declare_dram_parameter("xT", [C, N], BF16, isOutput=False)
    wq_d = nc.declare_dram_parameter("wq", [C, KC], BF16, isOutput=False)
    wk_d = nc.declare_dram_parameter("wk", [C, KC], BF16, isOutput=False)
    wv_d = nc.declare_dram_parameter("wv", [C, KC], BF16, isOutput=False)
    wo_d = nc.declare_dram_parameter("wo", [C, KC], BF16, isOutput=False)
    bb_d = nc.declare_dram_parameter("bb", [128, KC], F32, isOutput=False)
    y_d = nc.declare_dram_parameter("y", [N, KC], F32, isOutput=True)

    with tile.TileContext(nc) as tc:
        with (
            tc.tile_pool(name="wpool", bufs=1) as wpool,
            tc.tile_pool(name="xpool", bufs=1) as xpool,
            tc.tile_pool(name="seq", bufs=1) as seq,
            tc.tile_pool(name="work", bufs=3) as work,
            tc.tile_pool(name="small", bufs=2) as small,
            tc.tile_pool(name="psum", bufs=2, space="PSUM") as psum,
            tc.tile_pool(name="dram", bufs=1, space="DRAM") as dram,
        ):
            # ---- input DMAs (host supplies bf16) ----
            with nc.named_scope("load"):
                wq_sb = wpool.tile([128, CT, KC], BF16)
                wk_sb = wpool.tile([128, CT, KC], BF16)
                wv_sb = wpool.tile([128, CT, KC], BF16)
                wo_sb = wpool.tile([128, CT, KC], BF16)
                bb_sb = wpool.tile([128, KC], F32)
                xT_sb = xpool.tile([128, CT, N], BF16)
                # one strided DMA per tensor (issue cost ~600ns each on SP);
                # wq/wk/wv + first 512-col slice of xT land first so the
                # first q/k/v psum groups unblock early
                xT_src = xT_d.rearrange("(ct p) n -> p ct n", p=128)
                nc.sync.dma_start(wq_sb[:], wq_d.rearrange("(ct p) k -> p ct k", p=128))
                nc.sync.dma_start(xT_sb[:, :, 0:512], xT_src[:, :, 0:512])
                nc.sync.dma_start(wk_sb[:], wk_d.rearrange("(ct p) k -> p ct k", p=128))
                nc.sync.dma_start(wv_sb[:], wv_d.rearrange("(ct p) k -> p ct k", p=128))
                # x tail in 3 column chunks so v/k groups unblock progressively
                for c0, c1 in ((512, 1024), (1024, 1536), (1536, 2048)):
                    nc.sync.dma_start(xT_sb[:, :, c0:c1], xT_src[:, :, c0:c1])
                nc.sync.dma_start(wo_sb[:], wo_d.rearrange("(ct p) k -> p ct k", p=128))
                nc.sync.dma_start(bb_sb[:], bb_d[:])

            # ---- persistent tiles ----
            qT_sb = [seq.tile([128, N], BF16, name=f"qT{t}", tag=f"qT{t}") for t in range(3)]
            kT_sb = [seq.tile([128, N], BF16, name=f"kT{t}", tag=f"kT{t}") for t in range(3)]
            v_sb = seq.tile([128, NT * HPC * 65], BF16, tag="v")
            # ones column at offset 64 of every 65-wide block (softmax row-sum trick)
            nc.vector.memset(v_sb.rearrange("p (b s) -> p b s", s=65)[:, :, 64], 1.0)
            ao_sb = [seq.tile([128, N], BF16, name=f"ao{t}", tag=f"ao{t}") for t in range(3)]
            ones_sb = small.tile([1, 64], BF16, bufs=1)
            nc.vector.memset(ones_sb[:], 1.0)
            # AllGather bounce buffers: one per 512-column chunk for c0-c2;
            # the last chunk uses one gather per head-pair so the tail only
            # waits on hp2's small [128,512] collective
            ag_in = [dram.tile([KC, 512], BF16, name=f"ag_in{i}") for i in range(3)]
            ag_out = [dram.tile([C, 512], BF16, name=f"ag_out{i}") for i in range(3)]
            ag3_in = [dram.tile([128, 512], BF16, name=f"ag3_in{t}") for t in range(3)]
            ag3_out = [dram.tile([256, 512], BF16, name=f"ag3_out{t}") for t in range(3)]

            # ---- background work units (one PSUM group each) ----
            def qk_group(wsb, dst, hp, ni):
                # qT or kT for head-pair hp, columns ni*512:(ni+1)*512
                with nc.named_scope("qkv"):
                    qk_ps = psum.tile([128, 512], F32, name="qk_ps", tag="mm")
                    for ct in range(CT):
                        nc.tensor.matmul(
                            qk_ps[:],
                            wsb[:, ct, hp * 128:(hp + 1) * 128],
                            xT_sb[:, ct, ni * 512:(ni + 1) * 512],
                            start=(ct == 0), stop=(ct == CT - 1),
                        )
                    nc.vector.tensor_copy(dst[:, ni * 512:(ni + 1) * 512], qk_ps[:])

            def v_group(mj):
                with nc.named_scope("qkv"):
                    v_ps = psum.tile([128, KC], F32, name="v_ps", tag="mm")
                    for ct in range(CT):
                        nc.tensor.matmul(
                            v_ps[:],
                            xT_sb[:, ct, mj * 128:(mj + 1) * 128],
                            wv_sb[:, ct, :],
                            start=(ct == 0), stop=(ct == CT - 1),
                        )
                    for h in range(HPC):
                        nc.vector.tensor_copy(
                            v_sb[:, (mj * HPC + h) * 65:(mj * HPC + h) * 65 + 64],
                            v_ps[:, h * 64:(h + 1) * 64],
                        )

            aoF = {}  # chunk -> sbuf tile holding gathered aoT
            r_rows = {}  # (hp, c, i) -> stashed softmax row-sum row

            def proj_load(ci):
                # one strided DMA for the gathered chunk, issued from the
                # (idle at that point) scalar sequencer to dodge the busy
                # sync queue
                with nc.named_scope(f"proj{ci}"):
                    t = work.tile([128, CT, 512], BF16, name=f"aoF{ci}",
                                  tag="aoF", bufs=2)
                    src = ag_out[ci].rearrange("(kt p) n -> p kt n", p=128)
                    # two half DMAs: the first 3 kt contraction steps of each
                    # proj group unblock at half-transfer
                    nc.gpsimd.dma_start(t[:, 0:3, :], src[:, 0:3, :])
                    nc.gpsimd.dma_start(t[:, 3:CT, :], src[:, 3:CT, :])
                    aoF[ci] = t

            def proj_group(ci, njl):
                # one 128-row block of y within chunk ci's column window
                nj = ci * 4 + njl
                with nc.named_scope(f"proj{ci}"):
                    y_ps = psum.tile([128, KC], F32, name="y_ps", tag="mm")
                    kts = [0, 3, 1, 4, 2, 5] if ci == 3 else list(range(CT))
                    for j, kt in enumerate(kts):
                        nc.tensor.matmul(
                            y_ps[:],
                            aoF[ci][:, kt, njl * 128:(njl + 1) * 128],
                            wo_sb[:, kt, :],
                            start=(j == 0), stop=(j == CT - 1),
                        )
                    y_sb = work.tile([128, KC], F32, name="y_sb", tag="y")
                    nc.vector.tensor_add(y_sb[:], y_ps[:], bb_sb[:])
                    nc.gpsimd.dma_start(y_d[nj * 128:(nj + 1) * 128, :], y_sb[:])

            from collections import deque
            bg = deque()

            def drain_bg(n=1):
                for _ in range(n):
                    if bg:
                        bg.popleft()()

            def attn_chunk(hp, c, defer_norm=True):
                # attention for head-pair hp over query columns c*512:(c+1)*512.
                # The norm+ship block is deferred into the NEXT chunk's bg queue
                # (stashes first, freeing the oT banks) so the chunk seam never
                # serializes last-oT -> DVE chain -> rb matmul -> next S-pair.
                ci, coff = c, 0                     # ag chunk index / col offset
                col = c * 512
                with nc.named_scope(f"attn{c}"):
                    t = hp
                    kT_h, qT_h = kT_sb[t], qT_sb[t]
                    oT = [
                        psum.tile([65, 512], F32, name=f"oT{i}", tag="oT")
                        for i in range(2)
                    ]
                    for mj in range(NT):
                        drain_bg(1)
                        # both heads' S^T tiles share one 2-bank psum tile so a
                        # single 1024-wide ACT covers both heads' exp
                        sT = psum.tile([128, 1024], F32, name="sT", tag="sT", bufs=2)
                        for i in range(2):  # i = head within pair, PE row group i*64
                            po = i * 64
                            nc.tensor.matmul(
                                sT[:, i * 512:(i + 1) * 512],
                                kT_h[po:po + 64, mj * 128:(mj + 1) * 128],
                                qT_h[po:po + 64, col:col + 512],
                                start=True, stop=True,
                            )
                        pT = work.tile([128, 1024], BF16, name="pT", tag="pT", bufs=8)
                        nc.scalar.activation(
                            pT[:], sT[:], mybir.ActivationFunctionType.Exp, scale=SCALE,
                        )
                        for i in range(2):
                            h = hp * 2 + i
                            vblk = v_sb[:, (mj * HPC + h) * 65:(mj * HPC + h) * 65 + 65]
                            nc.tensor.matmul(
                                oT[i][:], vblk, pT[:, i * 512:(i + 1) * 512],
                                start=(mj == 0), stop=(mj == NT - 1),
                            )

                def stash_group():
                    # unnormalized output + row-sums out of PSUM (frees oT banks)
                    with nc.named_scope(f"attn{c}"):
                        for i in range(2):
                            po = i * 64
                            nc.vector.tensor_copy(
                                ao_sb[t][po:po + 64, col:col + 512], oT[i][0:64, :]
                            )
                            r_row = small.tile([1, 512], F32, name="r_row",
                                               tag="r_row", bufs=4)
                            nc.vector.tensor_copy(r_row[:], oT[i][64:65, :])
                            r_rows[(t, c, i)] = r_row

                def norm_group():
                    # reciprocal + K=1 broadcast matmul + in-place scale + ship
                    with nc.named_scope(f"attn{c}"):
                        for i in range(2):
                            po = i * 64
                            ao_slice = ao_sb[t][po:po + 64, col:col + 512]
                            rinv = small.tile([1, 512], F32, name="rinv", tag="rinv")
                            nc.vector.reciprocal_approx_fast(
                                rinv[:], r_rows.pop((t, c, i))[:]
                            )
                            rb_row = small.tile([1, 512], BF16, name="rb_row",
                                                tag="rb_row", bufs=4)
                            nc.vector.tensor_copy(rb_row[:], rinv[:])
                            rb_ps = psum.tile([64, 512], F32, name="rb_ps", tag="mm")
                            nc.tensor.matmul(rb_ps[:], ones_sb[:], rb_row[:],
                                             start=True, stop=True)
                            nc.vector.tensor_mul(ao_slice, ao_slice, rb_ps[:])
                            dst = (ag3_in[t][po:po + 64, 0:512] if c == 3 else
                                   ag_in[ci][t * 128 + po: t * 128 + po + 64,
                                             coff:coff + 512])
                            nc.gpsimd.dma_start(dst, ao_slice)

                if defer_norm:
                    bg.appendleft(norm_group)
                    bg.appendleft(stash_group)
                else:
                    stash_group()
                    norm_group()

            def emit_ag(ci):
                with nc.named_scope(f"ag{ci}"):
                    nc.gpsimd.collective_compute(
                        "AllGather",
                        mybir.AluOpType.bypass,
                        replica_groups=[[0, 1], [2, 3], [4, 5], [6, 7]],
                        ins=[ag_in[ci].opt()],
                        outs=[ag_out[ci].opt()],
                    )

            def emit_ag3(t):
                with nc.named_scope("ag3"):
                    nc.gpsimd.collective_compute(
                        "AllGather",
                        mybir.AluOpType.bypass,
                        replica_groups=[[0, 1], [2, 3], [4, 5], [6, 7]],
                        ins=[ag3_in[t].opt()],
                        outs=[ag3_out[t].opt()],
                    )

            def proj_load3(t):
                # hp t's gathered rows land at kt=t (head group 0) and kt=t+3
                with nc.named_scope("proj3"):
                    if 3 not in aoF:
                        aoF[3] = work.tile([128, CT, 512], BF16, name="aoF3",
                                           tag="aoF", bufs=2)
                    dst = aoF[3]
                    src3 = ag3_out[t].rearrange("(two p) n -> p two n", p=128)
                    nc.gpsimd.dma_start(dst[:, t, :], src3[:, 0, :])
                    nc.gpsimd.dma_start(dst[:, t + 3, :], src3[:, 1, :])

            # ---- emission schedule ----
            # Chunk-major, head-pair-rotating order: chunk c completes after
            # its hp=2 pass, so its AllGather fires ~2 chunks before the
            # dependent proj groups drain. Prologue covers hp0+hp1 k/q so the
            # rotation can start.
            def qg(hp, ni):
                return lambda: qk_group(wq_sb, qT_sb[hp], hp, ni)

            def kg(hp, ni):
                return lambda: qk_group(wk_sb, kT_sb[hp], hp, ni)

            qk_group(wq_sb, qT_sb[0], 0, 0)
            for ni in range(4):
                qk_group(wk_sb, kT_sb[0], 0, ni)
            v_group(0)
            v_group(1)
            for ni in range(4):
                qk_group(wk_sb, kT_sb[1], 1, ni)
            qk_group(wq_sb, qT_sb[1], 1, 0)

            bg.extend([lambda mj=mj: v_group(mj) for mj in range(2, NT)])
            attn_chunk(0, 0)
            bg.extend([kg(2, ni) for ni in range(4)] + [qg(2, 0)])
            attn_chunk(1, 0)
            bg.extend([qg(0, 1), qg(1, 1), qg(2, 1)])
            attn_chunk(2, 0)
            bg.extend([qg(0, 2), qg(1, 2), qg(2, 2)])
            attn_chunk(0, 1)
            # chunk c0's last ships were emitted by the deferred norm groups
            # during the chunk above, so the collective may only be emitted now
            emit_ag(0)
            proj_load(0)
            bg.extend([qg(0, 3), qg(1, 3), qg(2, 3)])
            attn_chunk(1, 1)
            attn_chunk(2, 1)
            attn_chunk(0, 2)
            emit_ag(1)
            proj_load(1)
            bg.extend([lambda njl=njl: proj_group(0, njl) for njl in range(4)])
            attn_chunk(1, 2)
            attn_chunk(2, 2)
            bg.extend([lambda njl=njl: proj_group(1, njl) for njl in range(4)])
            attn_chunk(0, 3)
            emit_ag(2)
            proj_load(2)
            attn_chunk(1, 3)
            # norm(0,3) was emitted during (1,3): hp0's gather can fire now
            emit_ag3(0)
            proj_load3(0)

            def ag3b_group():
                # hp1's ships land at iter 1 of the last chunk; firing its
                # gather from the bg queue at iter 2 hides it under the chunk
                emit_ag3(1)
                proj_load3(1)

            bg.append(ag3b_group)
            bg.extend([lambda njl=njl: proj_group(2, njl) for njl in range(4)])
            attn_chunk(2, 3, defer_norm=False)
            # tail: only hp2's small gather remains; proj(3)'s kt order
            # [0,3,1,4,2,5] lets its first 4 matmuls overlap this collective
            emit_ag3(2)
            proj_load3(2)
            for njl in range(4):
                proj_group(3, njl)

    nc.finalize()
    return nc


_NC = None
LAST_RESULTS = None


def _get_nc():
    global _NC
    if _NC is None:
        _NC = _build()
    return _NC


def kernel(x, w_qkv, w_out, b_out, _trace=False):
    global LAST_RESULTS
    nc = _get_nc()

    x = np.asarray(x, dtype=np.float32)
    w_qkv = np.asarray(w_qkv, dtype=np.float32)
    w_out = np.asarray(w_out, dtype=np.float32)
    b_out = np.asarray(b_out, dtype=np.float32)

    bf16 = ml_dtypes.bfloat16
    in_maps = []
    for c in range(8):
        b, g = c // 2, c % 2
        s = g * KC
        in_maps.append({
            "xT": np.ascontiguousarray(x[b].T).astype(bf16),
            "wq": np.ascontiguousarray(w_qkv[:, s:s + KC]).astype(bf16),
            "wk": np.ascontiguousarray(w_qkv[:, C + s:C + s + KC]).astype(bf16),
            "wv": np.ascontiguousarray(w_qkv[:, 2 * C + s:2 * C + s + KC]).astype(bf16),
            "wo": np.ascontiguousarray(w_out[:, s:s + KC]).astype(bf16),
            "bb": np.tile(b_out[s:s + KC], (128, 1)),
        })

    res = run_bass_kernel_spmd(nc, in_maps, core_ids=list(range(8)), trace=_trace)
    LAST_RESULTS = res

    out = np.empty((B, N, C), dtype=np.float32)
    for c in range(8):
        b, g = c // 2, c % 2
        out[b, :, g * KC:(g + 1) * KC] = res.results[c]["y"]
    return out


# revision 25
# speedup vs baseline: 1.0723x; 1.0723x over previous
"""Multi-head attention (B=4, N=2048, C=768, H=12) on 8 TRN2 NeuronCores.

Sharding: 4 batches x 2 head-groups (6 heads each); core = 2*b + g.
Per core:
  - qT/kT [64,2048] per head and v [2048,64] per head from host-pre-transposed xT
  - flash-style attention on transposed-S tiles:
      S^T(m,n) = kT.T @ qT   (PE, bf16, two heads paired in disjoint row groups)
      P^T = exp(S^T/8)       (ACT, -> bf16)
      o^T = [v|1].T @ P^T    (PE, bf16; ones column accumulates softmax row-sums)
  - normalize columns of o^T via reciprocal + K=1 broadcast matmul
  - AllGather normalized aoT (bf16) between pair cores, one collective per
    512-column chunk so each gather + projection hides under later attention
  - each core projects full aoT onto its half of w_out columns -> y [2048,384]

Scheduling: chunk-major, head-pair-rotating attention order. The qkv GEMM +
projection GEMM work is cut into single-PSUM-group units and drained one group
per attention mj-iteration ("background work"), so the PE stream stays dense
while ACT paces the attention inner loop. proj(ci) groups drain two chunks
after chunk ci's AllGather is emitted because ships lag the chunk end by the
norm chain and the collective waits on the partner's ships (~10us on HW).
Host only concatenates the 8 column-slices (no host math).
"""

import sys

sys.path.insert(0, "/opt/trn_rl_repo")

import ml_dtypes
import numpy as np

import concourse.bass as bass
import concourse.mybir as mybir
from concourse import bacc, tile
from concourse.bass_utils import run_bass_kernel_spmd

F32 = mybir.dt.float32
BF16 = mybir.dt.bfloat16

B, N, C, H, D = 4, 2048, 768, 12, 64
G = 2               # head groups (tensor-parallel dim)
HPC = H // G        # heads per core = 6
KC = HPC * D        # per-core head width = 384
CT = C // 128       # contraction tiles over C = 6
NT = N // 128       # 128-row seq tiles = 16
SCALE = D ** -0.5


def _build():
    nc = bacc.Bacc(None, num_devices=8)

    xT_d = nc.declare_dram_parameter("xT", [C, N], BF16, isOutput=False)
    wq_d = nc.declare_dram_parameter("wq", [C, KC], BF16, isOutput=False)
    wk_d = nc.declare_dram_parameter("wk", [C, KC], BF16, isOutput=False)
    wv_d = nc.declare_dram_parameter("wv", [C, KC], BF16, isOutput=False)
    wo_d = nc.declare_dram_parameter("wo", [C, KC], BF16, isOutput=False)
    bb_d = nc.declare_dram_parameter("bb", [128, KC], F32, isOutput=False)
    y_d = nc.declare_dram_parameter("y", [N, KC], F32, isOutput=True)

    with tile.TileContext(nc) as tc:
        with (
            tc.tile_pool(name="wpool", bufs=1) as wpool,
            tc.tile_pool(name="xpool", bufs=1) as xpool,
            tc.tile_pool(name="seq", bufs=1) as seq,
            tc.tile_pool(name="work", bufs=3) as work,
            tc.tile_pool(name="small", bufs=2) as small,
            tc.tile_pool(name="psum", bufs=2, space="PSUM") as psum,
            tc.tile_pool(name="dram", bufs=1, space="DRAM") as dram,
        ):
            # ---- input DMAs (host supplies bf16) ----
            with nc.named_scope("load"):
                wq_sb = wpool.tile([128, CT, KC], BF16)
                wk_sb = wpool.tile([128, CT, KC], BF16)
                wv_sb = wpool.tile([128, CT, KC], BF16)
                wo_sb = wpool.tile([128, CT, KC], BF16)
                bb_sb = wpool.tile([128, KC], F32)
                xT_sb = xpool.tile([128, CT, N], BF16)
                # one strided DMA per tensor (issue cost ~600ns each on SP);
                # wq/wk/wv + first 512-col slice of xT land first so the
                # first q/k/v psum groups unblock early
                xT_src = xT_d.rearrange("(ct p) n -> p ct n", p=128)
                nc.sync.dma_start(wq_sb[:], wq_d.rearrange("(ct p) k -> p ct k", p=128))
                nc.sync.dma_start(xT_sb[:, :, 0:512], xT_src[:, :, 0:512])
                nc.sync.dma_start(wk_sb[:], wk_d.rearrange("(ct p) k -> p ct k", p=128))
                nc.sync.dma_start(wv_sb[:], wv_d.rearrange("(ct p) k -> p ct k", p=128))
                # x tail in 3 column chunks so v/k groups unblock progressively
                for c0, c1 in ((512, 1024), (1024, 1536), (1536, 2048)):
                    nc.sync.dma_start(xT_sb[:, :, c0:c1], xT_src[:, :, c0:c1])
                nc.sync.dma_start(wo_sb[:], wo_d.rearrange("(ct p) k -> p ct k", p=128))
                nc.sync.dma_start(bb_sb[:], bb_d[:])

            # ---- persistent tiles ----
            qT_sb = [seq.tile([128, N], BF16, name=f"qT{t}", tag=f"qT{t}") for t in range(3)]
            kT_sb = [seq.tile([128, N], BF16, name=f"kT{t}", tag=f"kT{t}") for t in range(3)]
            v_sb = seq.tile([128, NT * HPC * 65], BF16, tag="v")
            # ones column at offset 64 of every 65-wide block (softmax row-sum trick)
            nc.vector.memset(v_sb.rearrange("p (b s) -> p b s", s=65)[:, :, 64], 1.0)
            ao_sb = [seq.tile([128, N], BF16, name=f"ao{t}", tag=f"ao{t}") for t in range(3)]
            ones_sb = small.tile([1, 64], BF16, bufs=1)
            nc.vector.memset(ones_sb[:], 1.0)
            # AllGather bounce buffers: one per 512-column chunk for c0-c2;
            # the last chunk uses one gather per head-pair so the tail only
            # waits on hp2's small [128,512] collective
            ag_in = [dram.tile([KC, 512], BF16, name=f"ag_in{i}") for i in range(3)]
            ag_out = [dram.tile([C, 512], BF16, name=f"ag_out{i}") for i in range(3)]
            ag3_in = [dram.tile([128, 512], BF16, name=f"ag3_in{t}") for t in range(3)]
            ag3_out = [dram.tile([256, 512], BF16, name=f"ag3_out{t}") for t in range(3)]

            # ---- background work units (one PSUM group each) ----
            def qk_group(wsb, dst, hp, ni):
                # qT or kT for head-pair hp, columns ni*512:(ni+1)*512
                with nc.named_scope("qkv"):
                    qk_ps = psum.tile([128, 512], F32, name="qk_ps", tag="mm")
                    for ct in range(CT):
                        nc.tensor.matmul(
                            qk_ps[:],
                            wsb[:, ct, hp * 128:(hp + 1) * 128],
                            xT_sb[:, ct, ni * 512:(ni + 1) * 512],
                            start=(ct == 0), stop=(ct == CT - 1),
                        )
                    nc.vector.tensor_copy(dst[:, ni * 512:(ni + 1) * 512], qk_ps[:])

            def v_group(mj):
                with nc.named_scope("qkv"):
                    v_ps = psum.tile([128, KC], F32, name="v_ps", tag="mm")
                    for ct in range(CT):
                        nc.tensor.matmul(
                            v_ps[:],
                            xT_sb[:, ct, mj * 128:(mj + 1) * 128],
                            wv_sb[:, ct, :],
                            start=(ct == 0), stop=(ct == CT - 1),
                        )
                    for h in range(HPC):
                        nc.vector.tensor_copy(
                            v_sb[:, (mj * HPC + h) * 65:(mj * HPC + h) * 65 + 64],
                            v_ps[:, h * 64:(h + 1) * 64],
                        )

            aoF = {}  # chunk -> sbuf tile holding gathered aoT
            r_rows = {}  # (hp, c, i) -> stashed softmax row-sum row

            def proj_load(ci):
                # one strided DMA for the gathered chunk, issued from the
                # (idle at that point) scalar sequencer to dodge the busy
                # sync queue
                with nc.named_scope(f"proj{ci}"):
                    t = work.tile([128, CT, 512], BF16, name=f"aoF{ci}",
                                  tag="aoF", bufs=2)
                    src = ag_out[ci].rearrange("(kt p) n -> p kt n", p=128)
                    # two half DMAs: the first 3 kt contraction steps of each
                    # proj group unblock at half-transfer
                    nc.gpsimd.dma_start(t[:, 0:3, :], src[:, 0:3, :])
                    nc.gpsimd.dma_start(t[:, 3:CT, :], src[:, 3:CT, :])
                    aoF[ci] = t

            def proj_group(ci, njl):
                # one 128-row block of y within chunk ci's column window
                nj = ci * 4 + njl
                with nc.named_scope(f"proj{ci}"):
                    y_ps = psum.tile([128, KC], F32, name="y_ps", tag="mm")
                    if ci == 3:
                        # hp-major order: the last two contraction steps only
                        # depend on hp2's (tail) gather
                        ops = [(aoF3[t][:, j], t + 3 * j) for t in range(3)
                               for j in range(2)]
                    else:
                        ops = [(aoF[ci][:, kt], kt) for kt in range(CT)]
                    for j, (lhs, kt) in enumerate(ops):
                        nc.tensor.matmul(
                            y_ps[:],
                            lhs[:, njl * 128:(njl + 1) * 128],
                            wo_sb[:, kt, :],
                            start=(j == 0), stop=(j == CT - 1),
                        )
                    y_sb = work.tile([128, KC], F32, name="y_sb", tag="y")
                    nc.vector.tensor_add(y_sb[:], y_ps[:], bb_sb[:])
                    nc.gpsimd.dma_start(y_d[nj * 128:(nj + 1) * 128, :], y_sb[:])

            from collections import deque
            bg = deque()

            def drain_bg(n=1):
                for _ in range(n):
                    if bg:
                        bg.popleft()()

            def attn_chunk(hp, c, defer_norm=True):
                # attention for head-pair hp over query columns c*512:(c+1)*512.
                # The norm+ship block is deferred into the NEXT chunk's bg queue
                # (stashes first, freeing the oT banks) so the chunk seam never
                # serializes last-oT -> DVE chain -> rb matmul -> next S-pair.
                ci, coff = c, 0                     # ag chunk index / col offset
                col = c * 512
                with nc.named_scope(f"attn{c}"):
                    t = hp
                    kT_h, qT_h = kT_sb[t], qT_sb[t]
                    oT = [
                        psum.tile([65, 512], F32, name=f"oT{i}", tag="oT")
                        for i in range(2)
                    ]
                    for mj in range(NT):
                        drain_bg(1)
                        # both heads' S^T tiles share one 2-bank psum tile so a
                        # single 1024-wide ACT covers both heads' exp
                        sT = psum.tile([128, 1024], F32, name="sT", tag="sT", bufs=2)
                        for i in range(2):  # i = head within pair, PE row group i*64
                            po = i * 64
                            nc.tensor.matmul(
                                sT[:, i * 512:(i + 1) * 512],
                                kT_h[po:po + 64, mj * 128:(mj + 1) * 128],
                                qT_h[po:po + 64, col:col + 512],
                                start=True, stop=True,
                            )
                        pT = work.tile([128, 1024], BF16, name="pT", tag="pT", bufs=8)
                        nc.scalar.activation(
                            pT[:], sT[:], mybir.ActivationFunctionType.Exp, scale=SCALE,
                        )
                        for i in range(2):
                            h = hp * 2 + i
                            vblk = v_sb[:, (mj * HPC + h) * 65:(mj * HPC + h) * 65 + 65]
                            nc.tensor.matmul(
                                oT[i][:], vblk, pT[:, i * 512:(i + 1) * 512],
                                start=(mj == 0), stop=(mj == NT - 1),
                            )

                def stash_group():
                    # unnormalized output + row-sums out of PSUM (frees oT banks)
                    with nc.named_scope(f"attn{c}"):
                        for i in range(2):
                            po = i * 64
                            nc.vector.tensor_copy(
                                ao_sb[t][po:po + 64, col:col + 512], oT[i][0:64, :]
                            )
                            r_row = small.tile([1, 512], F32, name="r_row",
                                               tag="r_row", bufs=4)
                            nc.vector.tensor_copy(r_row[:], oT[i][64:65, :])
                            r_rows[(t, c, i)] = r_row

                def norm_group():
                    # reciprocal + K=1 broadcast matmul + in-place scale + ship
                    with nc.named_scope(f"attn{c}"):
                        for i in range(2):
                            po = i * 64
                            ao_slice = ao_sb[t][po:po + 64, col:col + 512]
                            rinv = small.tile([1, 512], F32, name="rinv", tag="rinv")
                            nc.vector.reciprocal_approx_fast(
                                rinv[:], r_rows.pop((t, c, i))[:]
                            )
                            rb_row = small.tile([1, 512], BF16, name="rb_row",
                                                tag="rb_row", bufs=4)
                            nc.vector.tensor_copy(rb_row[:], rinv[:])
                            rb_ps = psum.tile([64, 512], F32, name="rb_ps", tag="mm")
                            nc.tensor.matmul(rb_ps[:], ones_sb[:], rb_row[:],
                                             start=True, stop=True)
                            nc.vector.tensor_mul(ao_slice, ao_slice, rb_ps[:])
                            dst = (ag3_in[t][po:po + 64, 0:512] if c == 3 else
                                   ag_in[ci][t * 128 + po: t * 128 + po + 64,
                                             coff:coff + 512])
                            nc.gpsimd.dma_start(dst, ao_slice)

                if defer_norm:
                    bg.appendleft(norm_group)
                    bg.appendleft(stash_group)
                else:
                    stash_group()
                    norm_group()

            def emit_ag(ci):
                with nc.named_scope(f"ag{ci}"):
                    nc.gpsimd.collective_compute(
                        "AllGather",
                        mybir.AluOpType.bypass,
                        replica_groups=[[0, 1], [2, 3], [4, 5], [6, 7]],
                        ins=[ag_in[ci].opt()],
                        outs=[ag_out[ci].opt()],
                    )

            def emit_ag3(t):
                with nc.named_scope("ag3"):
                    nc.gpsimd.collective_compute(
                        "AllGather",
                        mybir.AluOpType.bypass,
                        replica_groups=[[0, 1], [2, 3], [4, 5], [6, 7]],
                        ins=[ag3_in[t].opt()],
                        outs=[ag3_out[t].opt()],
                    )

            aoF3 = {}

            def proj_load3(t):
                # hp t's gathered rows land at kt=t (head group 0) and kt=t+3.
                # One SBUF tile per head-pair: Tile tracks deps per tile, so a
                # shared tile would make even kt=0 proj matmuls wait for the
                # last hp's gather DMA.
                with nc.named_scope("proj3"):
                    dst = work.tile([128, 2, 512], BF16, name=f"aoF3_{t}",
                                    tag="aoF3", bufs=3)
                    src3 = ag3_out[t].rearrange("(two p) n -> p two n", p=128)
                    nc.gpsimd.dma_start(dst[:], src3[:])
                    aoF3[t] = dst

            # ---- emission schedule ----
            # Chunk-major, head-pair-rotating order: chunk c completes after
            # its hp=2 pass, so its AllGather fires ~2 chunks before the
            # dependent proj groups drain. Prologue covers hp0+hp1 k/q so the
            # rotation can start.
            def qg(hp, ni):
                return lambda: qk_group(wq_sb, qT_sb[hp], hp, ni)

            def kg(hp, ni):
                return lambda: qk_group(wk_sb, kT_sb[hp], hp, ni)

            qk_group(wq_sb, qT_sb[0], 0, 0)
            for ni in range(4):
                qk_group(wk_sb, kT_sb[0], 0, ni)
            v_group(0)
            v_group(1)
            for ni in range(4):
                qk_group(wk_sb, kT_sb[1], 1, ni)
            qk_group(wq_sb, qT_sb[1], 1, 0)

            bg.extend([lambda mj=mj: v_group(mj) for mj in range(2, NT)])
            attn_chunk(0, 0)
            bg.extend([kg(2, ni) for ni in range(4)] + [qg(2, 0)])
            attn_chunk(1, 0)
            bg.extend([qg(0, 1), qg(1, 1), qg(2, 1)])
            attn_chunk(2, 0)
            bg.extend([qg(0, 2), qg(1, 2), qg(2, 2)])
            attn_chunk(0, 1)
            # chunk c0's last ships were emitted by the deferred norm groups
            # during the chunk above, so the collective may only be emitted now
            emit_ag(0)
            proj_load(0)
            bg.extend([qg(0, 3), qg(1, 3), qg(2, 3)])
            attn_chunk(1, 1)
            attn_chunk(2, 1)
            attn_chunk(0, 2)
            emit_ag(1)
            proj_load(1)
            bg.extend([lambda njl=njl: proj_group(0, njl) for njl in range(4)])
            attn_chunk(1, 2)
            attn_chunk(2, 2)
            bg.extend([lambda njl=njl: proj_group(1, njl) for njl in range(4)])
            attn_chunk(0, 3)
            emit_ag(2)
            proj_load(2)
            attn_chunk(1, 3)
            # norm(0,3) was emitted during (1,3): hp0's gather can fire now
            emit_ag3(0)
            proj_load3(0)
            bg.extend([lambda njl=njl: proj_group(2, njl) for njl in range(4)])
            attn_chunk(2, 3, defer_norm=False)
            emit_ag3(1)
            proj_load3(1)
            emit_ag3(2)
            proj_load3(2)
            for njl in range(4):
                proj_group(3, njl)

    nc.finalize()
    return nc


_NC = None
LAST_RESULTS = None


def _get_nc():
    global _NC
    if _NC is None:
        _NC = _build()
    return _NC


def kernel(x, w_qkv, w_out, b_out, _trace=False):
    global LAST_RESULTS
    nc = _get_nc()

    x = np.asarray(x, dtype=np.float32)
    w_qkv = np.asarray(w_qkv, dtype=np.float32)
    w_out = np.asarray(w_out, dtype=np.float32)
    b_out = np.asarray(b_out, dtype=np.float32)

    bf16 = ml_dtypes.bfloat16
    in_maps = []
    for c in range(8):
        b, g = c // 2, c % 2
        s = g * KC
        in_maps.append({
            "xT": np.ascontiguousarray(x[b].T).astype(bf16),
            "wq": np.ascontiguousarray(w_qkv[:, s:s + KC]).astype(bf16),
            "wk": np.ascontiguousarray(w_qkv[:, C + s:C + s + KC]).astype(bf16),
            "wv": np.ascontiguousarray(w_qkv[:, 2 * C + s:2 * C + s + KC]).astype(bf16),
            "wo": np.ascontiguousarray(w_out[:, s:s + KC]).astype(bf16),
            "bb": np.tile(b_out[s:s + KC], (128, 1)),
        })

    res = run_bass_kernel_spmd(nc, in_maps, core_ids=list(range(8)), trace=_trace)
    LAST_RESULTS = res

    out = np.empty((B, N, C), dtype=np.float32)
    for c in range(8):
        b, g = c // 2, c % 2
        out[b, :, g * KC:(g + 1) * KC] = res.results[c]["y"]
    return out


# revision 27
# speedup vs baseline: 1.2025x; 1.1215x over previous
"""Multi-head attention (B=4, N=2048, C=768, H=12) on 8 TRN2 NeuronCores.

Sharding: 4 batches x 2 head-groups (6 heads each); core = 2*b + g.
Per core:
  - qT/kT [64,2048] per head and v [2048,64] per head from host-pre-transposed xT
  - flash-style attention on transposed-S tiles:
      S^T(m,n) = kT.T @ qT   (PE, bf16, two heads paired in disjoint row groups)
      P^T = exp(S^T/8)       (ACT, -> bf16)
      o^T = [v|1].T @ P^T    (PE, bf16; ones column accumulates softmax row-sums)
  - normalize columns of o^T via reciprocal + K=1 broadcast matmul
  - AllGather normalized aoT (bf16) between pair cores, one collective per
    512-column chunk so each gather + projection hides under later attention
  - each core projects full aoT onto its half of w_out columns -> y [2048,384]

Scheduling: chunk-major, head-pair-rotating attention order. The qkv GEMM +
projection GEMM work is cut into single-PSUM-group units and drained one group
per attention mj-iteration ("background work"), so the PE stream stays dense
while ACT paces the attention inner loop. proj(ci) groups drain two chunks
after chunk ci's AllGather is emitted because ships lag the chunk end by the
norm chain and the collective waits on the partner's ships (~10us on HW).
Host only concatenates the 8 column-slices (no host math).
"""

import sys

sys.path.insert(0, "/opt/trn_rl_repo")

import ml_dtypes
import numpy as np

import concourse.bass as bass
import concourse.mybir as mybir
from concourse import bacc, tile
from concourse.bass_utils import run_bass_kernel_spmd

F32 = mybir.dt.float32
BF16 = mybir.dt.bfloat16

B, N, C, H, D = 4, 2048, 768, 12, 64
G = 2               # head groups (tensor-parallel dim)
HPC = H // G        # heads per core = 6
KC = HPC * D        # per-core head width = 384
CT = C // 128       # contraction tiles over C = 6
NT = N // 128       # 128-row seq tiles = 16
SCALE = D ** -0.5


def _build():
    nc = bacc.Bacc(None, num_devices=8)

    xT_d = nc.declare_dram_parameter("xT", [C, N], BF16, isOutput=False)
    wq_d = nc.declare_dram_parameter("wq", [C, KC], BF16, isOutput=False)
    wk_d = nc.declare_dram_parameter("wk", [C, KC], BF16, isOutput=False)
    wv_d = nc.declare_dram_parameter("wv", [C, KC], BF16, isOutput=False)
    wo_d = nc.declare_dram_parameter("wo", [C, KC], BF16, isOutput=False)
    bb_d = nc.declare_dram_parameter("bb", [128, KC], F32, isOutput=False)
    y_d = nc.declare_dram_parameter("y", [N, KC], F32, isOutput=True)

    with tile.TileContext(nc) as tc:
        with (
            tc.tile_pool(name="wpool", bufs=1) as wpool,
            tc.tile_pool(name="xpool", bufs=1) as xpool,
            tc.tile_pool(name="seq", bufs=1) as seq,
            tc.tile_pool(name="work", bufs=3) as work,
            tc.tile_pool(name="small", bufs=2) as small,
            tc.tile_pool(name="psum", bufs=2, space="PSUM") as psum,
            tc.tile_pool(name="dram", bufs=1, space="DRAM") as dram,
        ):
            # ---- input DMAs (host supplies bf16) ----
            with nc.named_scope("load"):
                wq_sb = wpool.tile([128, CT, KC], BF16)
                wk_sb = wpool.tile([128, CT, KC], BF16)
                wv_sb = wpool.tile([128, CT, KC], BF16)
                wo_sb = wpool.tile([128, CT, KC], BF16)
                bb_sb = wpool.tile([128, KC], F32)
                xT_sb = xpool.tile([128, CT, N], BF16)
                # one strided DMA per tensor (issue cost ~600ns each on SP);
                # wq/wk/wv + first 512-col slice of xT land first so the
                # first q/k/v psum groups unblock early
                xT_src = xT_d.rearrange("(ct p) n -> p ct n", p=128)
                nc.sync.dma_start(wq_sb[:], wq_d.rearrange("(ct p) k -> p ct k", p=128))
                nc.sync.dma_start(xT_sb[:, :, 0:512], xT_src[:, :, 0:512])
                nc.sync.dma_start(wk_sb[:], wk_d.rearrange("(ct p) k -> p ct k", p=128))
                nc.sync.dma_start(wv_sb[:], wv_d.rearrange("(ct p) k -> p ct k", p=128))
                # x tail in 3 column chunks so v/k groups unblock progressively
                for c0, c1 in ((512, 1024), (1024, 1536), (1536, 2048)):
                    nc.sync.dma_start(xT_sb[:, :, c0:c1], xT_src[:, :, c0:c1])
                nc.sync.dma_start(wo_sb[:], wo_d.rearrange("(ct p) k -> p ct k", p=128))
                nc.sync.dma_start(bb_sb[:], bb_d[:])

            # ---- persistent tiles ----
            qT_sb = [seq.tile([128, N], BF16, name=f"qT{t}", tag=f"qT{t}") for t in range(3)]
            kT_sb = [seq.tile([128, N], BF16, name=f"kT{t}", tag=f"kT{t}") for t in range(3)]
            v_sb = seq.tile([128, NT * HPC * 65], BF16, tag="v")
            # ones column at offset 64 of every 65-wide block (softmax row-sum trick)
            nc.vector.memset(v_sb.rearrange("p (b s) -> p b s", s=65)[:, :, 64], 1.0)
            ao_sb = [seq.tile([128, N], BF16, name=f"ao{t}", tag=f"ao{t}") for t in range(3)]
            ones_sb = small.tile([1, 64], BF16, bufs=1)
            nc.vector.memset(ones_sb[:], 1.0)
            # AllGather bounce buffers: one per 512-column chunk for c0-c2;
            # the last chunk uses one gather per head-pair so the tail only
            # waits on hp2's small [128,512] collective
            ag_in = [dram.tile([KC, 512], BF16, name=f"ag_in{i}") for i in range(3)]
            ag_out = [dram.tile([C, 512], BF16, name=f"ag_out{i}") for i in range(3)]
            ag3_in = [dram.tile([128, 512], BF16, name=f"ag3_in{t}") for t in range(3)]
            ag3_out = [dram.tile([256, 512], BF16, name=f"ag3_out{t}") for t in range(3)]

            # ---- background work units (one PSUM group each) ----
            def qk_group(wsb, dst, hp, ni):
                # qT or kT for head-pair hp, columns ni*512:(ni+1)*512
                with nc.named_scope("qkv"):
                    qk_ps = psum.tile([128, 512], F32, name="qk_ps", tag="mm")
                    for ct in range(CT):
                        nc.tensor.matmul(
                            qk_ps[:],
                            wsb[:, ct, hp * 128:(hp + 1) * 128],
                            xT_sb[:, ct, ni * 512:(ni + 1) * 512],
                            start=(ct == 0), stop=(ct == CT - 1),
                        )
                    nc.vector.tensor_copy(dst[:, ni * 512:(ni + 1) * 512], qk_ps[:])

            def v_group(mj):
                with nc.named_scope("qkv"):
                    v_ps = psum.tile([128, KC], F32, name="v_ps", tag="mm")
                    for ct in range(CT):
                        nc.tensor.matmul(
                            v_ps[:],
                            xT_sb[:, ct, mj * 128:(mj + 1) * 128],
                            wv_sb[:, ct, :],
                            start=(ct == 0), stop=(ct == CT - 1),
                        )
                    for h in range(HPC):
                        nc.vector.tensor_copy(
                            v_sb[:, (mj * HPC + h) * 65:(mj * HPC + h) * 65 + 64],
                            v_ps[:, h * 64:(h + 1) * 64],
                        )

            aoF = {}  # chunk -> sbuf tile holding gathered aoT
            r_rows = {}  # (hp, c, i) -> stashed softmax row-sum row

            def proj_load(ci):
                # one strided DMA for the gathered chunk, issued from the
                # (idle at that point) scalar sequencer to dodge the busy
                # sync queue
                with nc.named_scope(f"proj{ci}"):
                    t = work.tile([128, CT, 512], BF16, name=f"aoF{ci}",
                                  tag="aoF", bufs=2)
                    src = ag_out[ci].rearrange("(kt p) n -> p kt n", p=128)
                    # two half DMAs: the first 3 kt contraction steps of each
                    # proj group unblock at half-transfer
                    nc.gpsimd.dma_start(t[:, 0:3, :], src[:, 0:3, :])
                    nc.gpsimd.dma_start(t[:, 3:CT, :], src[:, 3:CT, :])
                    aoF[ci] = t

            def proj_group(ci, njl):
                # one 128-row block of y within chunk ci's column window
                nj = ci * 4 + njl
                with nc.named_scope(f"proj{ci}"):
                    y_ps = psum.tile([128, KC], F32, name="y_ps", tag="mm")
                    if ci == 3:
                        # hp-major: the last two contraction steps depend only
                        # on hp2's (tail) gather
                        ops = [(aoF3[t][:, j], t + 3 * j) for t in range(3)
                               for j in range(2)]
                    else:
                        ops = [(aoF[ci][:, kt], kt) for kt in range(CT)]
                    for j, (lhs, kt) in enumerate(ops):
                        nc.tensor.matmul(
                            y_ps[:],
                            lhs[:, njl * 128:(njl + 1) * 128],
                            wo_sb[:, kt, :],
                            start=(j == 0), stop=(j == CT - 1),
                        )
                    y_sb = work.tile([128, KC], F32, name="y_sb", tag="y")
                    nc.vector.tensor_add(y_sb[:], y_ps[:], bb_sb[:])
                    nc.gpsimd.dma_start(y_d[nj * 128:(nj + 1) * 128, :], y_sb[:])

            from collections import deque
            bg = deque()

            def drain_bg(n=1):
                for _ in range(n):
                    if bg:
                        bg.popleft()()

            def attn_chunk(hp, c, defer_norm=True):
                # attention for head-pair hp over query columns c*512:(c+1)*512.
                # The norm+ship block is deferred into the NEXT chunk's bg queue
                # (stashes first, freeing the oT banks) so the chunk seam never
                # serializes last-oT -> DVE chain -> rb matmul -> next S-pair.
                ci, coff = c, 0                     # ag chunk index / col offset
                col = c * 512
                with nc.named_scope(f"attn{c}"):
                    t = hp
                    kT_h, qT_h = kT_sb[t], qT_sb[t]
                    oT = [
                        psum.tile([65, 512], F32, name=f"oT{i}", tag="oT")
                        for i in range(2)
                    ]
                    for mj in range(NT):
                        drain_bg(1)
                        # both heads' S^T tiles share one 2-bank psum tile so a
                        # single 1024-wide ACT covers both heads' exp
                        sT = psum.tile([128, 1024], F32, name="sT", tag="sT", bufs=2)
                        for i in range(2):  # i = head within pair, PE row group i*64
                            po = i * 64
                            nc.tensor.matmul(
                                sT[:, i * 512:(i + 1) * 512],
                                kT_h[po:po + 64, mj * 128:(mj + 1) * 128],
                                qT_h[po:po + 64, col:col + 512],
                                start=True, stop=True,
                            )
                        pT = work.tile([128, 1024], BF16, name="pT", tag="pT", bufs=8)
                        nc.scalar.activation(
                            pT[:], sT[:], mybir.ActivationFunctionType.Exp, scale=SCALE,
                        )
                        for i in range(2):
                            h = hp * 2 + i
                            vblk = v_sb[:, (mj * HPC + h) * 65:(mj * HPC + h) * 65 + 65]
                            nc.tensor.matmul(
                                oT[i][:], vblk, pT[:, i * 512:(i + 1) * 512],
                                start=(mj == 0), stop=(mj == NT - 1),
                            )

                def stash_group():
                    # unnormalized output + row-sums out of PSUM (frees oT banks)
                    with nc.named_scope(f"attn{c}"):
                        for i in range(2):
                            po = i * 64
                            nc.vector.tensor_copy(
                                ao_sb[t][po:po + 64, col:col + 512], oT[i][0:64, :]
                            )
                            r_row = small.tile([1, 512], F32, name="r_row",
                                               tag="r_row", bufs=4)
                            nc.vector.tensor_copy(r_row[:], oT[i][64:65, :])
                            r_rows[(t, c, i)] = r_row

                def norm_group():
                    # reciprocal + K=1 broadcast matmul + in-place scale + ship
                    with nc.named_scope(f"attn{c}"):
                        for i in range(2):
                            po = i * 64
                            ao_slice = ao_sb[t][po:po + 64, col:col + 512]
                            rinv = small.tile([1, 512], F32, name="rinv", tag="rinv")
                            nc.vector.reciprocal_approx_fast(
                                rinv[:], r_rows.pop((t, c, i))[:]
                            )
                            rb_row = small.tile([1, 512], BF16, name="rb_row",
                                                tag="rb_row", bufs=4)
                            nc.vector.tensor_copy(rb_row[:], rinv[:])
                            rb_ps = psum.tile([64, 512], F32, name="rb_ps", tag="mm")
                            nc.tensor.matmul(rb_ps[:], ones_sb[:], rb_row[:],
                                             start=True, stop=True)
                            nc.vector.tensor_mul(ao_slice, ao_slice, rb_ps[:])
                            dst = (ag3_in[t][po:po + 64, 0:512] if c == 3 else
                                   ag_in[ci][t * 128 + po: t * 128 + po + 64,
                                             coff:coff + 512])
                            nc.gpsimd.dma_start(dst, ao_slice)

                if defer_norm:
                    bg.appendleft(norm_group)
                    bg.appendleft(stash_group)
                else:
                    stash_group()
                    norm_group()

            def emit_ag(ci):
                with nc.named_scope(f"ag{ci}"):
                    nc.gpsimd.collective_compute(
                        "AllGather",
                        mybir.AluOpType.bypass,
                        replica_groups=[[0, 1], [2, 3], [4, 5], [6, 7]],
                        ins=[ag_in[ci].opt()],
                        outs=[ag_out[ci].opt()],
                    )

            def emit_ag3(t):
                with nc.named_scope("ag3"):
                    nc.gpsimd.collective_compute(
                        "AllGather",
                        mybir.AluOpType.bypass,
                        replica_groups=[[0, 1], [2, 3], [4, 5], [6, 7]],
                        ins=[ag3_in[t].opt()],
                        outs=[ag3_out[t].opt()],
                    )

            aoF3 = {}

            def proj_load3(t):
                # hp t's gathered rows land at kt=t (head group 0) and kt=t+3.
                # Separate tiles (own tag each): Tile tracks deps per tile, so
                # a shared tile would make even the kt=0 proj matmuls wait for
                # the last hp's gather DMA.
                with nc.named_scope("proj3"):
                    dst = work.tile([128, 2, 512], BF16, name=f"aoF3_{t}",
                                    tag=f"aoF3_{t}", bufs=1)
                    src3 = ag3_out[t].rearrange("(two p) n -> p two n", p=128)
                    nc.gpsimd.dma_start(dst[:, 0, :], src3[:, 0, :])
                    nc.gpsimd.dma_start(dst[:, 1, :], src3[:, 1, :])
                    aoF3[t] = dst

            # ---- emission schedule ----
            # Chunk-major, head-pair-rotating order: chunk c completes after
            # its hp=2 pass, so its AllGather fires ~2 chunks before the
            # dependent proj groups drain. Prologue covers hp0+hp1 k/q so the
            # rotation can start.
            def qg(hp, ni):
                return lambda: qk_group(wq_sb, qT_sb[hp], hp, ni)

            def kg(hp, ni):
                return lambda: qk_group(wk_sb, kT_sb[hp], hp, ni)

            qk_group(wq_sb, qT_sb[0], 0, 0)
            for ni in range(4):
                qk_group(wk_sb, kT_sb[0], 0, ni)
            v_group(0)
            v_group(1)
            for ni in range(4):
                qk_group(wk_sb, kT_sb[1], 1, ni)
            qk_group(wq_sb, qT_sb[1], 1, 0)

            bg.extend([lambda mj=mj: v_group(mj) for mj in range(2, NT)])
            attn_chunk(0, 0)
            bg.extend([kg(2, ni) for ni in range(4)] + [qg(2, 0)])
            attn_chunk(1, 0)
            bg.extend([qg(0, 1), qg(1, 1), qg(2, 1)])
            attn_chunk(2, 0)
            bg.extend([qg(0, 2), qg(1, 2), qg(2, 2)])
            attn_chunk(0, 1)
            # chunk c0's last ships were emitted by the deferred norm groups
            # during the chunk above, so the collective may only be emitted now
            emit_ag(0)
            proj_load(0)
            bg.extend([qg(0, 3), qg(1, 3), qg(2, 3)])
            attn_chunk(1, 1)
            attn_chunk(2, 1)
            attn_chunk(0, 2)
            emit_ag(1)
            proj_load(1)
            bg.extend([lambda njl=njl: proj_group(0, njl) for njl in range(4)])
            attn_chunk(1, 2)
            attn_chunk(2, 2)
            bg.extend([lambda njl=njl: proj_group(1, njl) for njl in range(4)])
            attn_chunk(0, 3)
            emit_ag(2)
            proj_load(2)
            attn_chunk(1, 3)
            # norm(0,3) was emitted during (1,3): hp0's gather can fire now
            emit_ag3(0)
            proj_load3(0)
            bg.extend([lambda njl=njl: proj_group(2, njl) for njl in range(4)])
            attn_chunk(2, 3, defer_norm=False)
            emit_ag3(1)
            proj_load3(1)
            emit_ag3(2)
            proj_load3(2)
            for njl in range(4):
                proj_group(3, njl)

    nc.finalize()
    return nc


_NC = None
LAST_RESULTS = None


def _get_nc():
    global _NC
    if _NC is None:
        _NC = _build()
    return _NC


def kernel(x, w_qkv, w_out, b_out, _trace=False):
    global LAST_RESULTS
    nc = _get_nc()

    x = np.asarray(x, dtype=np.float32)
    w_qkv = np.asarray(w_qkv, dtype=np.float32)
    w_out = np.asarray(w_out, dtype=np.float32)
    b_out = np.asarray(b_out, dtype=np.float32)

    bf16 = ml_dtypes.bfloat16
    in_maps = []
    for c in range(8):
        b, g = c // 2, c % 2
        s = g * KC
        in_maps.append({
            "xT": np.ascontiguousarray(x[b].T).astype(bf16),
            "wq": np.ascontiguousarray(w_qkv[:, s:s + KC]).astype(bf16),
            "wk": np.ascontiguousarray(w_qkv[:, C + s:C + s + KC]).astype(bf16),
            "wv": np.ascontiguousarray(w_qkv[:, 2 * C + s:2 * C + s + KC]).astype(bf16),
            "wo": np.ascontiguousarray(w_out[:, s:s + KC]).astype(bf16),
            "bb": np.tile(b_out[s:s + KC], (128, 1)),
        })

    res = run_bass_kernel_spmd(nc, in_maps, core_ids=list(range(8)), trace=_trace)
    LAST_RESULTS = res

    out = np.empty((B, N, C), dtype=np.float32)
    for c in range(8):
        b, g = c // 2, c % 2
        out[b, :, g * KC:(g + 1) * KC] = res.results[c]["y"]
    return out


# revision 28
# speedup vs baseline: 1.2225x; 1.0166x over previous
"""Multi-head attention (B=4, N=2048, C=768, H=12) on 8 TRN2 NeuronCores.

Sharding: 4 batches x 2 head-groups (6 heads each); core = 2*b + g.
Per core:
  - qT/kT [64,2048] per head and v [2048,64] per head from host-pre-transposed xT
  - flash-style attention on transposed-S tiles:
      S^T(m,n) = kT.T @ qT   (PE, bf16, two heads paired in disjoint row groups)
      P^T = exp(S^T/8)       (ACT, -> bf16)
      o^T = [v|1].T @ P^T    (PE, bf16; ones column accumulates softmax row-sums)
  - normalize columns of o^T via reciprocal + K=1 broadcast matmul
  - AllGather normalized aoT (bf16) between pair cores, one collective per
    512-column chunk so each gather + projection hides under later attention
  - each core projects full aoT onto its half of w_out columns -> y [2048,384]

Scheduling: chunk-major, head-pair-rotating attention order. The qkv GEMM +
projection GEMM work is cut into single-PSUM-group units and drained one group
per attention mj-iteration ("background work"), so the PE stream stays dense
while ACT paces the attention inner loop. proj(ci) groups drain two chunks
after chunk ci's AllGather is emitted because ships lag the chunk end by the
norm chain and the collective waits on the partner's ships (~10us on HW).
Host only concatenates the 8 column-slices (no host math).
"""

import sys

sys.path.insert(0, "/opt/trn_rl_repo")

import ml_dtypes
import numpy as np

import concourse.bass as bass
import concourse.mybir as mybir
from concourse import bacc, tile
from concourse.bass_utils import run_bass_kernel_spmd

F32 = mybir.dt.float32
BF16 = mybir.dt.bfloat16

B, N, C, H, D = 4, 2048, 768, 12, 64
G = 2               # head groups (tensor-parallel dim)
HPC = H // G        # heads per core = 6
KC = HPC * D        # per-core head width = 384
CT = C // 128       # contraction tiles over C = 6
NT = N // 128       # 128-row seq tiles = 16
SCALE = D ** -0.5


def _build():
    nc = bacc.Bacc(None, num_devices=8)

    xT_d = nc.declare_dram_parameter("xT", [C, N], BF16, isOutput=False)
    wq_d = nc.declare_dram_parameter("wq", [C, KC], BF16, isOutput=False)
    wk_d = nc.declare_dram_parameter("wk", [C, KC], BF16, isOutput=False)
    wv_d = nc.declare_dram_parameter("wv", [C, KC], BF16, isOutput=False)
    wo_d = nc.declare_dram_parameter("wo", [C, KC], BF16, isOutput=False)
    bb_d = nc.declare_dram_parameter("bb", [128, KC], F32, isOutput=False)
    y_d = nc.declare_dram_parameter("y", [N, KC], F32, isOutput=True)

    with tile.TileContext(nc) as tc:
        with (
            tc.tile_pool(name="wpool", bufs=1) as wpool,
            tc.tile_pool(name="xpool", bufs=1) as xpool,
            tc.tile_pool(name="seq", bufs=1) as seq,
            tc.tile_pool(name="work", bufs=3) as work,
            tc.tile_pool(name="small", bufs=2) as small,
            tc.tile_pool(name="psum", bufs=2, space="PSUM") as psum,
            tc.tile_pool(name="dram", bufs=1, space="DRAM") as dram,
        ):
            # ---- input DMAs (host supplies bf16) ----
            with nc.named_scope("load"):
                wq_sb = wpool.tile([128, CT, KC], BF16)
                wk_sb = wpool.tile([128, CT, KC], BF16)
                wv_sb = wpool.tile([128, CT, KC], BF16)
                wo_sb = wpool.tile([128, CT, KC], BF16)
                bb_sb = wpool.tile([128, KC], F32)
                xT_sb = xpool.tile([128, CT, N], BF16)
                # one strided DMA per tensor (issue cost ~600ns each on SP);
                # wq/wk/wv + first 512-col slice of xT land first so the
                # first q/k/v psum groups unblock early
                xT_src = xT_d.rearrange("(ct p) n -> p ct n", p=128)
                nc.sync.dma_start(wq_sb[:], wq_d.rearrange("(ct p) k -> p ct k", p=128))
                nc.sync.dma_start(xT_sb[:, :, 0:512], xT_src[:, :, 0:512])
                nc.sync.dma_start(wk_sb[:], wk_d.rearrange("(ct p) k -> p ct k", p=128))
                nc.sync.dma_start(wv_sb[:], wv_d.rearrange("(ct p) k -> p ct k", p=128))
                # x tail in 3 column chunks so v/k groups unblock progressively
                for c0, c1 in ((512, 1024), (1024, 1536), (1536, 2048)):
                    nc.sync.dma_start(xT_sb[:, :, c0:c1], xT_src[:, :, c0:c1])
                nc.sync.dma_start(wo_sb[:], wo_d.rearrange("(ct p) k -> p ct k", p=128))
                nc.sync.dma_start(bb_sb[:], bb_d[:])

            # ---- persistent tiles ----
            qT_sb = [seq.tile([128, N], BF16, name=f"qT{t}", tag=f"qT{t}") for t in range(3)]
            kT_sb = [seq.tile([128, N], BF16, name=f"kT{t}", tag=f"kT{t}") for t in range(3)]
            v_sb = seq.tile([128, NT * HPC * 65], BF16, tag="v")
            # ones column at offset 64 of every 65-wide block (softmax row-sum trick)
            nc.vector.memset(v_sb.rearrange("p (b s) -> p b s", s=65)[:, :, 64], 1.0)
            ao_sb = [seq.tile([128, N], BF16, name=f"ao{t}", tag=f"ao{t}") for t in range(3)]
            ones_sb = small.tile([1, 64], BF16, bufs=1)
            nc.vector.memset(ones_sb[:], 1.0)
            # AllGather bounce buffers: one per 512-column chunk for c0-c2;
            # the last chunk uses one gather per head-pair so the tail only
            # waits on hp2's small [128,512] collective
            ag_in = [dram.tile([KC, 512], BF16, name=f"ag_in{i}") for i in range(3)]
            ag_out = [dram.tile([C, 512], BF16, name=f"ag_out{i}") for i in range(3)]
            ag3_in = [dram.tile([128, 512], BF16, name=f"ag3_in{t}") for t in range(3)]
            ag3_out = [dram.tile([256, 512], BF16, name=f"ag3_out{t}") for t in range(3)]

            # ---- background work units (one PSUM group each) ----
            def qk_group(wsb, dst, hp, ni):
                # qT or kT for head-pair hp, columns ni*512:(ni+1)*512
                with nc.named_scope("qkv"):
                    qk_ps = psum.tile([128, 512], F32, name="qk_ps", tag="mm")
                    for ct in range(CT):
                        nc.tensor.matmul(
                            qk_ps[:],
                            wsb[:, ct, hp * 128:(hp + 1) * 128],
                            xT_sb[:, ct, ni * 512:(ni + 1) * 512],
                            start=(ct == 0), stop=(ct == CT - 1),
                        )
                    nc.vector.tensor_copy(dst[:, ni * 512:(ni + 1) * 512], qk_ps[:])

            def v_group(mj):
                with nc.named_scope("qkv"):
                    v_ps = psum.tile([128, KC], F32, name="v_ps", tag="mm")
                    for ct in range(CT):
                        nc.tensor.matmul(
                            v_ps[:],
                            xT_sb[:, ct, mj * 128:(mj + 1) * 128],
                            wv_sb[:, ct, :],
                            start=(ct == 0), stop=(ct == CT - 1),
                        )
                    for h in range(HPC):
                        nc.vector.tensor_copy(
                            v_sb[:, (mj * HPC + h) * 65:(mj * HPC + h) * 65 + 64],
                            v_ps[:, h * 64:(h + 1) * 64],
                        )

            aoF = {}  # chunk -> sbuf tile holding gathered aoT
            r_rows = {}  # (hp, c, i) -> stashed softmax row-sum row

            def proj_load(ci):
                # one strided DMA for the gathered chunk, issued from the
                # (idle at that point) scalar sequencer to dodge the busy
                # sync queue
                with nc.named_scope(f"proj{ci}"):
                    t = work.tile([128, CT, 512], BF16, name=f"aoF{ci}",
                                  tag="aoF", bufs=2)
                    src = ag_out[ci].rearrange("(kt p) n -> p kt n", p=128)
                    # two half DMAs: the first 3 kt contraction steps of each
                    # proj group unblock at half-transfer
                    nc.gpsimd.dma_start(t[:, 0:3, :], src[:, 0:3, :])
                    nc.gpsimd.dma_start(t[:, 3:CT, :], src[:, 3:CT, :])
                    aoF[ci] = t

            def proj_group(ci, njl):
                # one 128-row block of y within chunk ci's column window
                nj = ci * 4 + njl
                with nc.named_scope(f"proj{ci}"):
                    y_ps = psum.tile([128, KC], F32, name="y_ps", tag="mm")
                    if ci == 3:
                        # hp-major: the last two contraction steps depend only
                        # on hp2's (tail) gather
                        ops = [(aoF3[t][:, j], t + 3 * j) for t in range(3)
                               for j in range(2)]
                    else:
                        ops = [(aoF[ci][:, kt], kt) for kt in range(CT)]
                    for j, (lhs, kt) in enumerate(ops):
                        nc.tensor.matmul(
                            y_ps[:],
                            lhs[:, njl * 128:(njl + 1) * 128],
                            wo_sb[:, kt, :],
                            start=(j == 0), stop=(j == CT - 1),
                        )
                    y_sb = work.tile([128, KC], F32, name="y_sb", tag="y")
                    nc.vector.tensor_add(y_sb[:], y_ps[:], bb_sb[:])
                    nc.gpsimd.dma_start(y_d[nj * 128:(nj + 1) * 128, :], y_sb[:])

            from collections import deque
            bg = deque()

            def drain_bg(n=1):
                for _ in range(n):
                    if bg:
                        bg.popleft()()

            def attn_chunk(hp, c, defer_norm=True):
                # attention for head-pair hp over query columns c*512:(c+1)*512.
                # The norm+ship block is deferred into the NEXT chunk's bg queue
                # (stashes first, freeing the oT banks) so the chunk seam never
                # serializes last-oT -> DVE chain -> rb matmul -> next S-pair.
                ci, coff = c, 0                     # ag chunk index / col offset
                col = c * 512
                with nc.named_scope(f"attn{c}"):
                    t = hp
                    kT_h, qT_h = kT_sb[t], qT_sb[t]
                    oT = [
                        psum.tile([65, 512], F32, name=f"oT{i}", tag="oT")
                        for i in range(2)
                    ]
                    for mj in range(NT):
                        drain_bg(1)
                        # both heads' S^T tiles share one 2-bank psum tile so a
                        # single 1024-wide ACT covers both heads' exp
                        sT = psum.tile([128, 1024], F32, name="sT", tag="sT", bufs=2)
                        for i in range(2):  # i = head within pair, PE row group i*64
                            po = i * 64
                            nc.tensor.matmul(
                                sT[:, i * 512:(i + 1) * 512],
                                kT_h[po:po + 64, mj * 128:(mj + 1) * 128],
                                qT_h[po:po + 64, col:col + 512],
                                start=True, stop=True,
                            )
                        pT = work.tile([128, 1024], BF16, name="pT", tag="pT", bufs=8)
                        nc.scalar.activation(
                            pT[:], sT[:], mybir.ActivationFunctionType.Exp, scale=SCALE,
                        )
                        for i in range(2):
                            h = hp * 2 + i
                            vblk = v_sb[:, (mj * HPC + h) * 65:(mj * HPC + h) * 65 + 65]
                            nc.tensor.matmul(
                                oT[i][:], vblk, pT[:, i * 512:(i + 1) * 512],
                                start=(mj == 0), stop=(mj == NT - 1),
                            )

                def stash_group():
                    # unnormalized output + row-sums out of PSUM (frees oT banks)
                    with nc.named_scope(f"attn{c}"):
                        for i in range(2):
                            po = i * 64
                            nc.vector.tensor_copy(
                                ao_sb[t][po:po + 64, col:col + 512], oT[i][0:64, :]
                            )
                            r_row = small.tile([1, 512], F32, name="r_row",
                                               tag="r_row", bufs=4)
                            nc.vector.tensor_copy(r_row[:], oT[i][64:65, :])
                            r_rows[(t, c, i)] = r_row

                def norm_group():
                    # reciprocal + K=1 broadcast matmul + in-place scale + ship
                    with nc.named_scope(f"attn{c}"):
                        for i in range(2):
                            po = i * 64
                            ao_slice = ao_sb[t][po:po + 64, col:col + 512]
                            rinv = small.tile([1, 512], F32, name="rinv", tag="rinv")
                            nc.vector.reciprocal_approx_fast(
                                rinv[:], r_rows.pop((t, c, i))[:]
                            )
                            rb_row = small.tile([1, 512], BF16, name="rb_row",
                                                tag="rb_row", bufs=4)
                            nc.vector.tensor_copy(rb_row[:], rinv[:])
                            rb_ps = psum.tile([64, 512], F32, name="rb_ps", tag="mm")
                            nc.tensor.matmul(rb_ps[:], ones_sb[:], rb_row[:],
                                             start=True, stop=True)
                            nc.vector.tensor_mul(ao_slice, ao_slice, rb_ps[:])
                            dst = (ag3_in[t][po:po + 64, 0:512] if c == 3 else
                                   ag_in[ci][t * 128 + po: t * 128 + po + 64,
                                             coff:coff + 512])
                            nc.gpsimd.dma_start(dst, ao_slice)

                if defer_norm:
                    bg.appendleft(norm_group)
                    bg.appendleft(stash_group)
                else:
                    stash_group()
                    norm_group()

            def emit_ag(ci):
                with nc.named_scope(f"ag{ci}"):
                    nc.gpsimd.collective_compute(
                        "AllGather",
                        mybir.AluOpType.bypass,
                        replica_groups=[[0, 1], [2, 3], [4, 5], [6, 7]],
                        ins=[ag_in[ci].opt()],
                        outs=[ag_out[ci].opt()],
                    )

            def emit_ag3(t):
                with nc.named_scope("ag3"):
                    nc.gpsimd.collective_compute(
                        "AllGather",
                        mybir.AluOpType.bypass,
                        replica_groups=[[0, 1], [2, 3], [4, 5], [6, 7]],
                        ins=[ag3_in[t].opt()],
                        outs=[ag3_out[t].opt()],
                    )

            aoF3 = {}

            def proj_load3(t):
                # hp t's gathered rows land at kt=t (head group 0) and kt=t+3.
                # Separate tiles (own tag each): Tile tracks deps per tile, so
                # a shared tile would make even the kt=0 proj matmuls wait for
                # the last hp's gather DMA.
                with nc.named_scope("proj3"):
                    dst = work.tile([128, 2, 512], BF16, name=f"aoF3_{t}",
                                    tag=f"aoF3_{t}", bufs=1)
                    src3 = ag3_out[t].rearrange("(two p) n -> p two n", p=128)
                    nc.gpsimd.dma_start(dst[:, 0, :], src3[:, 0, :])
                    nc.gpsimd.dma_start(dst[:, 1, :], src3[:, 1, :])
                    aoF3[t] = dst

            # ---- emission schedule ----
            # Chunk-major, head-pair-rotating order: chunk c completes after
            # its hp=2 pass, so its AllGather fires ~2 chunks before the
            # dependent proj groups drain. Prologue covers hp0+hp1 k/q so the
            # rotation can start.
            def qg(hp, ni):
                return lambda: qk_group(wq_sb, qT_sb[hp], hp, ni)

            def kg(hp, ni):
                return lambda: qk_group(wk_sb, kT_sb[hp], hp, ni)

            qk_group(wq_sb, qT_sb[0], 0, 0)
            for ni in range(4):
                qk_group(wk_sb, kT_sb[0], 0, ni)
            v_group(0)
            v_group(1)
            for ni in range(4):
                qk_group(wk_sb, kT_sb[1], 1, ni)
            qk_group(wq_sb, qT_sb[1], 1, 0)

            bg.extend([lambda mj=mj: v_group(mj) for mj in range(2, NT)])
            attn_chunk(0, 0)
            bg.extend([kg(2, ni) for ni in range(4)] + [qg(2, 0)])
            attn_chunk(1, 0)
            bg.extend([qg(0, 1), qg(1, 1), qg(2, 1)])
            attn_chunk(2, 0)
            bg.extend([qg(0, 2), qg(1, 2), qg(2, 2)])
            attn_chunk(0, 1)
            # chunk c0's last ships were emitted by the deferred norm groups
            # during the chunk above, so the collective may only be emitted now
            emit_ag(0)
            proj_load(0)
            bg.extend([qg(0, 3), qg(1, 3), qg(2, 3)])
            attn_chunk(1, 1)
            attn_chunk(2, 1)
            attn_chunk(0, 2)
            emit_ag(1)
            proj_load(1)
            bg.extend([lambda njl=njl: proj_group(0, njl) for njl in range(4)])
            attn_chunk(1, 2)
            attn_chunk(2, 2)
            bg.extend([lambda njl=njl: proj_group(1, njl) for njl in range(4)])
            attn_chunk(0, 3)
            emit_ag(2)
            proj_load(2)
            attn_chunk(1, 3)
            # norm(0,3) was emitted during (1,3): hp0's gather can fire now
            emit_ag3(0)
            proj_load3(0)
            bg.extend([lambda njl=njl: proj_group(2, njl) for njl in range(4)])
            attn_chunk(2, 3, defer_norm=False)
            emit_ag3(1)
            proj_load3(1)
            emit_ag3(2)
            proj_load3(2)
            # tail projection: all four 128-row groups accumulate their
            # hp0/hp1 contraction steps first (PSUM slots freed by the
            # finished attention), so only the 8 hp2-dependent matmuls wait
            # on the final collective. Same per-group accumulation order
            # [0,3,1,4,2,5] as before.
            with nc.named_scope("proj3"):
                y3 = []
                for njl in range(4):
                    y3.append(psum.tile([128, KC], F32, name=f"y3_{njl}",
                                        tag="sT" if njl < 2 else "oT"))
                for njl in range(4):
                    for j, (t, jj) in enumerate(((0, 0), (0, 1), (1, 0), (1, 1))):
                        nc.tensor.matmul(
                            y3[njl][:],
                            aoF3[t][:, jj, njl * 128:(njl + 1) * 128],
                            wo_sb[:, t + 3 * jj, :],
                            start=(j == 0), stop=False,
                        )
                for njl in range(4):
                    for j, (t, jj) in enumerate(((2, 0), (2, 1))):
                        nc.tensor.matmul(
                            y3[njl][:],
                            aoF3[t][:, jj, njl * 128:(njl + 1) * 128],
                            wo_sb[:, t + 3 * jj, :],
                            start=False, stop=(j == 1),
                        )
                    y_sb = work.tile([128, KC], F32, name="y_sb", tag="y")
                    nc.vector.tensor_add(y_sb[:], y3[njl][:], bb_sb[:])
                    nc.gpsimd.dma_start(
                        y_d[(12 + njl) * 128:(13 + njl) * 128, :], y_sb[:]
                    )

    nc.finalize()
    return nc


_NC = None
LAST_RESULTS = None


def _get_nc():
    global _NC
    if _NC is None:
        _NC = _build()
    return _NC


def kernel(x, w_qkv, w_out, b_out, _trace=False):
    global LAST_RESULTS
    nc = _get_nc()

    x = np.asarray(x, dtype=np.float32)
    w_qkv = np.asarray(w_qkv, dtype=np.float32)
    w_out = np.asarray(w_out, dtype=np.float32)
    b_out = np.asarray(b_out, dtype=np.float32)

    bf16 = ml_dtypes.bfloat16
    in_maps = []
    for c in range(8):
        b, g = c // 2, c % 2
        s = g * KC
        in_maps.append({
            "xT": np.ascontiguousarray(x[b].T).astype(bf16),
            "wq": np.ascontiguousarray(w_qkv[:, s:s + KC]).astype(bf16),
            "wk": np.ascontiguousarray(w_qkv[:, C + s:C + s + KC]).astype(bf16),
            "wv": np.ascontiguousarray(w_qkv[:, 2 * C + s:2 * C + s + KC]).astype(bf16),
            "wo": np.ascontiguousarray(w_out[:, s:s + KC]).astype(bf16),
            "bb": np.tile(b_out[s:s + KC], (128, 1)),
        })

    res = run_bass_kernel_spmd(nc, in_maps, core_ids=list(range(8)), trace=_trace)
    LAST_RESULTS = res

    out = np.empty((B, N, C), dtype=np.float32)
    for c in range(8):
        b, g = c // 2, c % 2
        out[b, :, g * KC:(g + 1) * KC] = res.results[c]["y"]
    return out
